# revision 11
# baseline (speedup 1.0000x reference)
"""Max-dilated conv2d kernel for Trainium2 (Bass/Tile), 8-core data parallel.

out[b,oc,oh,ow] = max_{ic,kh,kw} x[b,ic,oh+2*kh, ow+2*kw] * w[oc,ic,kh,kw]

Shapes (hardcoded): x (8,32,68,68) f32, w (32,32,3,3) f32, out (8,32,64,64) f32.
stride=1, dilation=2.

Sharding: batch across the 8 NeuronCores (1 image per core), weights replicated.

mode="tri" (default) — three-engine fp16 pipeline:
  Partition layout p = icq*32 + oc (icq in 0..3, oc in 0..31); the 32 input
  channels form 8 groups of 4 (ic = h*4 + icq).  x is converted to fp16 and
  replicated across the 32 oc partitions ON THE HOST, so the device just
  streams a contiguous [128, 8, 68, 68] fp16 tensor from DRAM (9.2 MB).
  Per (h, kh, kw) plane the work acc = max(acc, x_shifted * w) is split by
  engine at stock-instruction rates:
    - DVE self planes:  tensor_scalar_mul fp16 (4x mode, 0.26 ns/el) into a
      tmp, then tensor_tensor max fp16 (2x mode, 0.52 ns/el) into accD.
    - Act planes: ScalarE computes the product (0.83 ns/el); DVE tensor_max
      folds it into accD.
    - GpSimd planes: ScalarE computes the product; GpSimd tensor_tensor max
      folds it into accG.
  4/2/3 planes per group balance the three engines at ~17 us/group each.
  A cross-partition tree-max (128->64->32, SBUF DMA realign + tensor_max)
  reduces the 4 icq slots; out is written fp16 and cast to fp32 on the host.

mode="fp32"/"mixed" — the previous generation kernel (exact / scalar-offload),
kept for A/B comparison.
"""

import sys

sys.path.insert(0, "/opt/trn_rl_repo")

import numpy as np

import concourse.bacc as bacc
import concourse.tile as tile
from concourse import mybir
from concourse import bass_utils

IC, OC, K = 32, 32, 3
H = W = 68
OH = OW = 64
DH = DW = 2
NCORES = 8
NGROUPS = 8  # ic groups of 4
PLANES = NGROUPS * K * K  # 72
F32 = mybir.dt.float32
F16 = mybir.dt.float16

MODE = "tri"
# mixed mode: how many of the 9 planes per group stay on the exact fp32
# fused-stt path (the rest go ScalarE-fp16-product + VectorE fp16 max)
STT_PER_GROUP = [3, 2, 3, 2, 3, 2, 3, 2]

# tri mode per-group plane routing (k = kh*3+kw in 0..8):
#   D: DVE tensor_scalar_mul + tensor_max   A: ScalarE mul -> DVE max
#   G: ScalarE mul -> GpSimd max
TRI_D = (0, 2, 4, 6)
TRI_A = (7, 8)
TRI_G = (1, 3, 5)

_cache: dict = {}


def _build_tri():
    nc = bacc.Bacc("TRN2", debug=False, num_devices=NCORES, num_swdge_queues=4)
    xr_d = nc.dram_tensor("xr", [128, NGROUPS, H, W], F16, kind="ExternalInput").ap()
    wv32_d = nc.dram_tensor("wv32", [128, PLANES], F32, kind="ExternalInput").ap()
    wv16_d = nc.dram_tensor("wv16", [128, PLANES], F16, kind="ExternalInput").ap()
    out_d = nc.dram_tensor("out", [OC, OH, OW], F16, kind="ExternalOutput").ap()

    amax = mybir.AluOpType.max
    amul = mybir.AluOpType.mult

    # plane routing per group: g planes -> GpSimd product (folded on DVE one
    # group later so the slow Pool engine never blocks the fold stream),
    # a -> DVE TS self, rest -> ScalarE product.  All folds on DVE via NCH
    # round-robin accumulator chains (hides the serial TT write-ack gap).
    G_PER = 2
    A_CNT = [2, 1, 2, 1, 2, 1, 2, 1]  # 12

    with tile.TileContext(nc) as tc:
        with (
            tc.tile_pool(name="const", bufs=1) as cpool,
            tc.tile_pool(name="xbuf", bufs=1) as xpool,
            tc.tile_pool(name="pd", bufs=2) as pdpool,
            tc.tile_pool(name="pa", bufs=5) as papool,
            tc.tile_pool(name="pg", bufs=5) as pgpool,
            tc.tile_pool(name="work", bufs=1) as wpool,
        ):
            wv32 = cpool.tile([128, PLANES], F32, tag="wv32")
            wv16 = cpool.tile([128, PLANES], F16, tag="wv16")
            # split the weight loads so their 128 tiny per-partition
            # descriptors spread across queues instead of serializing.
            for s4 in range(4):
                p0, p1 = s4 * 32, (s4 + 1) * 32
                nc.sync.dma_start(wv32[p0:p1, :], wv32_d[p0:p1, :])
                nc.scalar.dma_start(wv16[p0:p1, :], wv16_d[p0:p1, :])

            # per-group x tiles so consumers wait only on their own group's
            # load, not the whole 9.2 MB stream.
            xg = [
                xpool.tile([128, H, W], F16, tag="xr%d" % h, name="xr%d" % h)
                for h in range(NGROUPS)
            ]
            # group 0 in 4 row chunks dispatched from 3 engines in parallel;
            # remaining groups from gpsimd (cheap software DGE).
            chunk_eng = [nc.sync, nc.scalar, nc.sync, nc.gpsimd]
            for s in range(4):
                r0, r1 = s * 17, (s + 1) * 17
                chunk_eng[s].dma_start(xg[0][:, r0:r1, :], xr_d[:, 0, r0:r1, :])
            for h in range(1, NGROUPS):
                nc.gpsimd.dma_start(xg[h][:, :, :], xr_d[:, h])

            NCH = 3
            chains = [
                wpool.tile([128, OH, OW], F16, tag="acc%d" % c, name="acc%d" % c)
                for c in range(NCH)
            ]
            chain_live = [False] * NCH
            pending: list = [None] * NCH
            rr = [0]

            def fold(prod_ap):
                c = rr[0] % NCH
                rr[0] += 1
                if not chain_live[c]:
                    if pending[c] is None:
                        pending[c] = prod_ap
                    else:
                        nc.vector.tensor_max(chains[c][:], pending[c], prod_ap)
                        pending[c] = None
                        chain_live[c] = True
                else:
                    nc.vector.tensor_max(chains[c][:], chains[c][:], prod_ap)

            def viewof(h, k):
                kh, kw = divmod(k, K)
                return xg[h][:, DH * kh : DH * kh + OH, DW * kw : DW * kw + OW]

            gp_queue: list = []
            for h in range(NGROUPS):
                g, a = G_PER, A_CNT[h]
                ks = list(range(K * K))
                g_ks, a_ks, b_ks = ks[:g], ks[g : g + a], ks[g + a :]
                base = h * (K * K)

                for k in g_ks:
                    j = base + k
                    prod = pgpool.tile([128, OH, OW], F16, tag="pg")
                    nc.gpsimd.tensor_tensor(
                        prod[:],
                        viewof(h, k),
                        wv16[:, j : j + 1].broadcast_to([128, OH, OW]),
                        amul,
                    )
                    gp_queue.append(prod)

                act_prods = []
                for k in b_ks:
                    j = base + k
                    prod = papool.tile([128, OH, OW], F16, tag="pa")
                    nc.scalar.mul(prod[:], viewof(h, k), wv32[:, j : j + 1])
                    act_prods.append(prod)

                # DVE: previous group's gpsimd products first (surely ready),
                # then self planes, then this group's ScalarE products.
                if h > 0:
                    ready = gp_queue[: len(gp_queue) - G_PER]
                    del gp_queue[: len(gp_queue) - G_PER]
                else:
                    ready = []
                for prod in ready[:1]:
                    fold(prod[:])
                for k in a_ks:
                    j = base + k
                    c = rr[0] % NCH
                    if not chain_live[c] and pending[c] is None:
                        rr[0] += 1
                        nc.vector.tensor_scalar_mul(
                            chains[c][:], viewof(h, k), wv32[:, j : j + 1]
                        )
                        chain_live[c] = True
                    else:
                        prod = pdpool.tile([128, OH, OW], F16, tag="pd")
                        nc.vector.tensor_scalar_mul(
                            prod[:], viewof(h, k), wv32[:, j : j + 1]
                        )
                        fold(prod[:])
                for prod in ready[1:]:
                    fold(prod[:])
                for prod in act_prods:
                    fold(prod[:])

            for prod in gp_queue:
                fold(prod[:])
            gp_queue.clear()

            # drain: merge chains into chains[0]
            for c in range(1, NCH):
                assert chain_live[c] and pending[c] is None
                nc.vector.tensor_max(chains[0][:], chains[0][:], chains[c][:])

            # cross-partition tree-max in two pixel halves; reuse chains[1]
            # (dead) as the 64-partition staging and chains[2] as the output
            # staging to save SBUF.
            acc, t64, o32 = chains[0], chains[1], chains[2]
            for hi, (a, b) in enumerate([(0, 32), (32, 64)]):
                for s in range(2):
                    r0 = a + s * 16
                    r1 = r0 + 16
                    eng = nc.gpsimd if s else nc.sync
                    eng.dma_start(t64[0:64, r0:r1, :], acc[64:128, r0:r1, :])
                nc.vector.tensor_max(
                    t64[0:64, a:b, :], t64[0:64, a:b, :], acc[0:64, a:b, :]
                )
                eng = nc.gpsimd if hi else nc.sync
                eng.dma_start(o32[0:32, a:b, :], t64[32:64, a:b, :])
                nc.vector.tensor_max(
                    o32[0:32, a:b, :], o32[0:32, a:b, :], t64[0:32, a:b, :]
                )
                for s in range(2):
                    r0 = a + s * 16
                    r1 = r0 + 16
                    eng = nc.gpsimd if s else nc.sync
                    eng.dma_start(out_d[:, r0:r1, :], o32[0:32, r0:r1, :])

    nc.compile()
    return nc


def _build_legacy(mode: str):
    """Previous-generation kernel (fp32 exact / mixed scalar-offload)."""
    nc = bacc.Bacc("TRN2", debug=False, num_devices=NCORES)
    x_d = nc.dram_tensor("x", [IC, H, W], F32, kind="ExternalInput").ap()
    wv_d = nc.dram_tensor("wv", [128, PLANES], F32, kind="ExternalInput").ap()
    out_d = nc.dram_tensor("out", [OC, OH, OW], F32, kind="ExternalOutput").ap()

    mult = mybir.AluOpType.mult
    amax = mybir.AluOpType.max

    with tile.TileContext(nc) as tc:
        with (
            tc.tile_pool(name="const", bufs=1) as cpool,
            tc.tile_pool(name="xrep", bufs=4) as xpool,
            tc.tile_pool(name="work", bufs=1) as wpool,
        ):
            wv_sb = cpool.tile([128, PLANES], F32, tag="wv")
            nc.sync.dma_start(wv_sb[:, :], wv_d[:, :])

            acc_v = wpool.tile([128, OH, OW], F32, tag="acc_v")
            acc_h = (
                wpool.tile([128, OH, OW], F16, tag="acc_h", name="acc_h")
                if mode == "mixed"
                else None
            )

            dma_engines = (
                [nc.sync, nc.scalar, nc.gpsimd]
                if mode == "fp32"
                else [nc.sync, nc.gpsimd]
            )
            first_v = True
            first_h = True
            ei = 0
            for h in range(NGROUPS):
                xr = xpool.tile([128, H, W], F32, tag="xr")
                if h == 0:
                    for s in range(4):
                        r0, r1 = s * 17, (s + 1) * 17
                        for icq in range(4):
                            src = (
                                x_d[h * 4 + icq]
                                .unsqueeze(0)
                                .broadcast_to([32, H, W])
                            )
                            dma_engines[ei % len(dma_engines)].dma_start(
                                xr[icq * 32 : (icq + 1) * 32, r0:r1],
                                src[:, r0:r1],
                            )
                            ei += 1
                else:
                    for icq in range(4):
                        src = (
                            x_d[h * 4 + icq].unsqueeze(0).broadcast_to([32, H, W])
                        )
                        for s in range(2):
                            r0, r1 = s * 34, (s + 1) * 34
                            dma_engines[ei % len(dma_engines)].dma_start(
                                xr[icq * 32 : (icq + 1) * 32, r0:r1],
                                src[:, r0:r1],
                            )
                            ei += 1

                n_stt = K * K if mode == "fp32" else STT_PER_GROUP[h]
                last = h == NGROUPS - 1
                if last:
                    splits = [(0, 32), (32, 64)]
                elif h == 0:
                    splits = None
                else:
                    splits = [(0, 64)]

                for k in range(K * K):
                    kh, kw = divmod(k, K)
                    j = h * (K * K) + k
                    wcol = wv_sb[:, j : j + 1]
                    on_stt = k >= K * K - n_stt
                    if h == 0:
                        if k < 3:
                            ksplits = [(0, 13), (13, 30), (30, 47), (47, 64)]
                        elif k < 6:
                            ksplits = [(0, 30), (30, 64)]
                        else:
                            ksplits = [(0, 64)]
                    else:
                        ksplits = splits
                    for a, b in ksplits:
                        view = xr[
                            :, DH * kh + a : DH * kh + b, DW * kw : DW * kw + OW
                        ]
                        if on_stt:
                            accw = acc_v[:, a:b, :]
                            if first_v:
                                nc.vector.tensor_scalar_mul(accw, view, wcol)
                            else:
                                nc.vector.scalar_tensor_tensor(
                                    accw, view, wcol, accw, mult, amax
                                )
                        else:
                            acch = acc_h[:, a:b, :]
                            if first_h:
                                nc.scalar.mul(acch, view, wcol)
                            else:
                                prod = xpool.tile(
                                    [128, b - a, OW], F16, tag="prod", name="prod", bufs=6
                                )
                                nc.scalar.mul(prod[:], view, wcol)
                                nc.vector.tensor_max(acch, acch, prod[:])
                    if on_stt:
                        first_v = False
                    else:
                        first_h = False

            t64 = wpool.tile([64, OH, OW], F32, tag="t64")
            out_sb = wpool.tile([32, OH, OW], F32, tag="out_sb")
            for hi, (a, b) in enumerate([(0, 32), (32, 64)]):
                if mode == "mixed":
                    nc.vector.tensor_max(
                        acc_v[:, a:b, :], acc_v[:, a:b, :], acc_h[:, a:b, :]
                    )
                for s in range(2):
                    r0 = a + s * 16
                    r1 = r0 + 16
                    dma_engines[(hi + s) % len(dma_engines)].dma_start(
                        t64[:, r0:r1, :], acc_v[64:128, r0:r1, :]
                    )
                nc.vector.tensor_max(
                    t64[:, a:b, :], t64[:, a:b, :], acc_v[0:64, a:b, :]
                )
                dma_engines[hi % len(dma_engines)].dma_start(
                    out_sb[:, a:b, :], t64[32:64, a:b, :]
                )
                nc.vector.tensor_max(
                    out_sb[:, a:b, :], out_sb[:, a:b, :], t64[0:32, a:b, :]
                )
                for s in range(2):
                    r0 = a + s * 16
                    r1 = r0 + 16
                    dma_engines[(hi + s) % len(dma_engines)].dma_start(
                        out_d[:, r0:r1, :], out_sb[:, r0:r1, :]
                    )

    nc.compile()
    return nc


def _build(mode: str = MODE):
    if mode in _cache:
        return _cache[mode]
    nc = _build_tri() if mode == "tri" else _build_legacy(mode)
    _cache[mode] = nc
    return nc


def _make_wv(w: np.ndarray) -> np.ndarray:
    """wv[p, h*9+k] = w[p%32, h*4 + p//32, kh, kw] with k = kh*3+kw."""
    wr = w.reshape(OC, NGROUPS, 4, K * K)  # (oc, h, icq, k); ic = h*4+icq
    wv = wr.transpose(2, 0, 1, 3).reshape(4 * OC, PLANES)  # (icq*32+oc, h*9+k)
    return np.ascontiguousarray(wv, dtype=np.float32)


def _make_xrep(x16_b: np.ndarray) -> np.ndarray:
    """[128, 8, 68, 68] fp16 with xrep[icq*32+oc, h] = x16_b[h*4+icq]."""
    xr = x16_b.reshape(NGROUPS, 4, H, W)  # (h, icq, H, W)
    rep = np.broadcast_to(
        xr.transpose(1, 0, 2, 3)[:, None], (4, OC, NGROUPS, H, W)
    ).reshape(128, NGROUPS, H, W)
    return np.ascontiguousarray(rep)


def _ensure_axon_hooks_module():
    """bass_utils imports antenv.axon_hooks when tracing is requested (e.g.
    via BASS_TRACE).  The module is absent on this image; provide a stub so
    the run degrades to untraced instead of crashing."""
    try:
        import antenv.axon_hooks  # noqa: F401
    except Exception:
        import types

        mod = types.ModuleType("antenv.axon_hooks")
        mod._hook = None
        mod.get_axon_ntff_profile_hook = lambda: getattr(mod, "_hook", None)
        mod.set_axon_ntff_profile_hook = lambda h: setattr(mod, "_hook", h)
        sys.modules["antenv.axon_hooks"] = mod
        try:
            import antenv

            antenv.axon_hooks = mod
        except Exception:
            pass


def _make_in_maps(x: np.ndarray, w: np.ndarray, mode: str):
    wv = _make_wv(w)
    if mode == "tri":
        x16 = x.astype(np.float16)
        wv16 = wv.astype(np.float16)
        return [
            {"xr": _make_xrep(x16[b]), "wv32": wv, "wv16": wv16}
            for b in range(x.shape[0])
        ]
    return [{"x": x[b], "wv": wv} for b in range(x.shape[0])]


def kernel(x, weight, stride_h=1, stride_w=1, dilation_h=2, dilation_w=2):
    _ensure_axon_hooks_module()
    x = np.ascontiguousarray(np.asarray(x, dtype=np.float32))
    w = np.ascontiguousarray(np.asarray(weight, dtype=np.float32))
    assert int(stride_h) == 1 and int(stride_w) == 1
    assert int(dilation_h) == DH and int(dilation_w) == DW
    B = x.shape[0]
    assert x.shape == (B, IC, H, W) and w.shape == (OC, IC, K, K)
    assert B == NCORES

    nc = _build(MODE)
    in_maps = _make_in_maps(x, w, MODE)
    res = bass_utils.run_bass_kernel_spmd(nc, in_maps, core_ids=list(range(B)))
    out = np.stack([r["out"] for r in res.results], axis=0)
    return out.astype(np.float32)


def run_traced(x, weight, mode=MODE, **trace_kwargs):
    """Like kernel() but with hardware profiling; returns (out, BassKernelResults)."""
    x = np.ascontiguousarray(np.asarray(x, dtype=np.float32))
    w = np.ascontiguousarray(np.asarray(weight, dtype=np.float32))
    nc = _build(mode)
    in_maps = _make_in_maps(x, w, mode)
    res = bass_utils.run_bass_kernel_spmd(
        nc, in_maps, core_ids=list(range(x.shape[0])), trace=True, **trace_kwargs
    )
    out = np.stack([r["out"] for r in res.results], axis=0)
    return out.astype(np.float32), res
